# revision 1
# baseline (speedup 1.0000x reference)
"""EGNN encoder kernel for Trainium2 (Bass/Tile), 8-core SPMD — v2.

Design (v2, degree-major):
  - Nodes relabeled by dest-degree (descending), interleaved across the 8
    cores so every core and every 128-row block has a near-identical degree
    profile -> identical shapes across cores (single SPMD NEFF) and minimal
    slot padding.
  - Edge layout is degree-major: partition = dest node (within its block),
    free dim = edge slot. Segment sums become strided free-dim vector
    reduces; dest-side features are free stride-0 broadcasts. No one-hot
    matmuls, no dest-side gather.
  - One dma_gather per block from tabB (128B rows: B-projection bf16 x32 +
    coord f32 x3). Each 256B descriptor covers TWO table rows, idx=col>>1
    fits int16 without range splitting; a parity select picks the right
    half on-chip. Gathers round-robin over 4 SWDGE queues (~4x descriptor
    throughput).
  - Edge MLP feat-major via batched PE transposes + block-diagonal weights
    (4 edge-slot groups per 128-wide matmul), activations on ACT with fused
    bias, 512-wide batches.
"""

import math
import os
from contextlib import ExitStack
from dataclasses import dataclass, field

import numpy as np

import concourse.bass as bass
import concourse.tile as tile
from concourse import mybir
from concourse.bass import AP
from concourse.masks import make_identity

F32 = mybir.dt.float32
BF16 = mybir.dt.bfloat16
I32 = mybir.dt.int32
I16 = mybir.dt.int16
ALU = mybir.AluOpType
ACTF = mybir.ActivationFunctionType

NC = 8        # cores
H = 32        # hidden
NL = 4        # layers
NQ = 4        # SWDGE gather queues


@dataclass
class Cfg:
    N: int
    E: int
    G: int
    L: float = 10.0
    EPS: float = 1e-8
    ROWN: int = 0
    BBLK: int = 0
    RPAD: int = 0
    NPAD: int = 0
    S_list: list = field(default_factory=list)
    CUM: list = field(default_factory=list)
    TOT: int = 0
    CHUNKS: list = field(default_factory=list)
    wslots: dict = field(default_factory=dict)
    WC: int = 0

    def derive_static(self):
        self.ROWN = self.N // NC
        self.BBLK = (self.ROWN + 127) // 128
        self.RPAD = self.BBLK * 128
        self.NPAD = ((self.N + 127) // 128) * 128
        self.CHUNKS = []
        o = 0
        while o < self.RPAD:
            w = min(512, self.RPAD - o)
            self.CHUNKS.append((o, w))
            o += w


# ---------------------------------------------------------------- host pre

def preprocess(inp, cfg: Cfg):
    cfg.derive_static()
    N, E, G = cfg.N, cfg.E, cfg.G
    ROWN, BBLK = cfg.ROWN, cfg.BBLK

    row = np.asarray(inp["edge_index"][0]).astype(np.int64)
    col = np.asarray(inp["edge_index"][1]).astype(np.int64)
    pos = np.asarray(inp["pos"]).astype(np.float32)
    x_in = np.asarray(inp["x"]).astype(np.float32).reshape(-1)
    batch = np.asarray(inp["batch"]).astype(np.int64)

    # ---- degree-sorted relabeling, interleaved across cores ----
    deg = np.bincount(row, minlength=N)
    order = np.argsort(-deg, kind="stable")      # rank k -> orig node id
    k = np.arange(N)
    newid_of_rank = (k % NC) * ROWN + (k // NC)
    perm = np.empty(N, np.int64)
    perm[order] = newid_of_rank                  # orig -> new
    row_n = perm[row]
    col_n = perm[col]
    pos_n = np.empty_like(pos)
    pos_n[perm] = pos
    x_n = np.empty_like(x_in)
    x_n[perm] = x_in
    batch_n = np.empty_like(batch)
    batch_n[perm] = batch
    deg_n = np.zeros(N, np.int64)
    deg_n[perm] = deg
    deg_sorted = deg[order]                      # descending

    # shared slot counts per block: S_b = deg of the first (max-degree) rank
    # in the block across all cores, rounded up to a multiple of 4
    S_list = []
    for b in range(BBLK):
        d = int(deg_sorted[min(b * 128 * NC, N - 1)])
        S_list.append(max(4, ((d + 3) // 4) * 4))
    cfg.S_list = S_list
    cfg.CUM = np.concatenate([[0], np.cumsum(S_list)]).astype(np.int64).tolist()
    cfg.TOT = int(cfg.CUM[-1])
    TOT = cfg.TOT

    # ---- per-edge slot assignment (global, then split per core) ----
    eorder = np.argsort(row_n, kind="stable")
    rs, cs = row_n[eorder], col_n[eorder]
    node_start = np.searchsorted(rs, np.arange(N))
    j_all = np.arange(E, dtype=np.int64) - node_start[rs]

    gcnt = np.bincount(batch, minlength=G).astype(np.float64)
    invg = (1.0 / np.maximum(gcnt, 1.0)).astype(np.float32)

    per_core = []
    for c in range(NC):
        lo = np.searchsorted(rs, c * ROWN)
        hi = np.searchsorted(rs, (c + 1) * ROWN)
        r_loc = rs[lo:hi] - c * ROWN
        c_gl = cs[lo:hi]
        jj = j_all[lo:hi]
        bb = r_loc // 128
        pp = r_loc % 128

        idxc = np.zeros((16, TOT * 8), np.int16)     # compact wrap16 idx
        pself = np.zeros((128, TOT), np.float32)     # parity (0/1)
        maskf = np.zeros((128, TOT), np.float32)     # 1 real / 0 pad
        for b in range(BBLK):
            S = S_list[b]
            m = bb == b
            jb, pb, cb = jj[m], pp[m], c_gl[m]
            assert jb.max(initial=-1) < S, (c, b, S, jb.max(initial=-1))
            flat = np.zeros(S * 128, np.int16)       # pad idx -> 0
            flat[jb * 128 + pb] = (cb >> 1).astype(np.int16)
            idxc[:, cfg.CUM[b] * 8:cfg.CUM[b + 1] * 8] = \
                flat.reshape(S * 8, 16).T
            pself[pb, cfg.CUM[b] + jb] = (cb & 1).astype(np.float32)
            maskf[pb, cfg.CUM[b] + jb] = 1.0

        d = {}
        d["idxc"] = idxc
        d["pself"] = pself
        d["maskf"] = maskf
        cnt = deg_n[c * ROWN:(c + 1) * ROWN].astype(np.float64)
        inv = np.zeros(cfg.RPAD, np.float32)
        inv[:ROWN] = (1.0 / np.maximum(cnt, 1.0)).astype(np.float32)
        d["invc"] = inv.reshape(BBLK, 128).T.copy()
        p4 = np.zeros((cfg.RPAD, 4), np.float32)
        p4[:ROWN, :3] = pos_n[c * ROWN:(c + 1) * ROWN]
        d["posnm"] = p4.reshape(BBLK, 128, 4).transpose(1, 0, 2).reshape(
            128, BBLK * 4).copy()
        x2 = np.zeros((2, cfg.RPAD), np.float32)
        x2[0, :ROWN] = x_n[c * ROWN:(c + 1) * ROWN]
        x2[1, :] = 1.0
        d["x2"] = x2
        goh = np.zeros((cfg.RPAD, G), np.float32)
        bloc = batch_n[c * ROWN:(c + 1) * ROWN]
        goh[np.arange(ROWN), bloc] = invg[bloc]
        d["goh"] = goh.reshape(BBLK, 128, G).transpose(1, 0, 2).reshape(
            128, BBLK * G).copy()
        per_core.append(d)

    # ---- weight pack (shared across cores) --------------------------------
    slots = {}
    colp = [0]
    wrows = []

    def add(name, arr):
        arr = np.asarray(arr, np.float32)
        assert arr.ndim == 2 and arr.shape[0] <= 128
        slots[name] = (arr.shape[0], colp[0], arr.shape[1])
        colp[0] += arr.shape[1]
        wrows.append(arr)

    def blkdiag(w, n=4):
        kk, m = w.shape
        out = np.zeros((n * kk, n * m), np.float32)
        for i in range(n):
            out[i * kk:(i + 1) * kk, i * m:(i + 1) * m] = w
        return out

    g = lambda kname: np.asarray(inp[kname], np.float32)
    add("EMB", np.vstack([g("emb_in_w"), g("emb_in_b")[None, :]]))
    for l in range(NL):
        w1 = g("edge_w1")[l]; b1 = g("edge_b1")[l]
        ab = np.zeros((33, 64), np.float32)
        ab[:32, :32] = w1[0:32]; ab[32, :32] = b1
        ab[:32, 32:] = w1[32:64]
        add(f"AB{l}", ab)
        add(f"w1c{l}", np.tile(w1[64:65, :], (128, 1)))
        add(f"W2{l}", blkdiag(g("edge_w2")[l]))
        add(f"b2{l}", np.tile(g("edge_b2")[l], 4)[:, None])
        add(f"C1{l}", blkdiag(g("coord_w1")[l]))
        add(f"c1{l}", np.tile(g("coord_b1")[l], 4)[:, None])
        add(f"C2{l}", blkdiag(g("coord_w2")[l]))
        add(f"c2{l}", np.full((128, 1), float(g("coord_b2")[l][0]), np.float32))
        add(f"N1h{l}", np.vstack([g("node_w1")[l][0:32], g("node_b1")[l][None, :]]))
        add(f"N1m{l}", g("node_w1")[l][32:64])
        add(f"N2{l}", np.vstack([g("node_w2")[l], g("node_b2")[l][None, :]]))
    add("EOUT", np.vstack([g("emb_out_w"), g("emb_out_b")[None, :]]))
    add("ZMU", np.vstack([g("zmu_w"), g("zmu_b")[None, :]]))
    add("ZSIG", np.vstack([g("zsig_w"), g("zsig_b")[None, :]]))
    cfg.WC = colp[0]
    cfg.wslots = slots
    wpack = np.zeros((128, cfg.WC), np.float32)
    c0 = 0
    for arr in wrows:
        wpack[:arr.shape[0], c0:c0 + arr.shape[1]] = arr
        c0 += arr.shape[1]

    for d in per_core:
        d["wpack"] = wpack
    return per_core


def input_specs(cfg: Cfg):
    BBLK, RPAD, G, TOT = cfg.BBLK, cfg.RPAD, cfg.G, cfg.TOT
    return {
        "idxc": ((16, TOT * 8), np.int16),
        "pself": ((128, TOT), np.float32),
        "maskf": ((128, TOT), np.float32),
        "invc": ((128, BBLK), np.float32),
        "posnm": ((128, BBLK * 4), np.float32),
        "x2": ((2, RPAD), np.float32),
        "goh": ((128, BBLK * G), np.float32),
        "wpack": ((128, cfg.WC), np.float32),
    }


# ---------------------------------------------------------------- builder

def build(tc, outs, ins, cfg: Cfg):
    nc = tc.nc
    BBLK, RPAD, NPAD, G = cfg.BBLK, cfg.RPAD, cfg.NPAD, cfg.G
    CHUNKS, S_list, CUM, TOT = cfg.CHUNKS, cfg.S_list, cfg.CUM, cfg.TOT
    SMAX = max(S_list)
    L = cfg.L

    mu_o, sig_o = outs["mu"], outs["sig"]

    tabBo = nc.dram_tensor("tabBo", [RPAD, 32], F32, kind="Internal")
    tabB = nc.dram_tensor("tabB", [NPAD, 32], F32, kind="Internal",
                          addr_space="Shared")
    idxfull = nc.dram_tensor("idxfull", [128, TOT * 8], I16, kind="Internal")
    gsin = nc.dram_tensor("gsin", [32, G], F32, kind="Internal")
    gsout = nc.dram_tensor("gsout", [32, G], F32, kind="Internal",
                           addr_space="Shared")

    # gather source view: two 128B rows per 256B element
    gsrc = AP(tabB, 0, [[64, NPAD // 2], [1, 64]])

    ctx = ExitStack()
    with ctx:
        wp = ctx.enter_context(tc.tile_pool(name="wp", bufs=1))

        # ---------------- persistent state ----------------
        wt = wp.tile([128, cfg.WC], F32, tag="wt")
        nc.sync.dma_start(wt[:], ins["wpack"][:])

        def W(name):
            p, c0, w = cfg.wslots[name]
            return wt[0:p, c0:c0 + w]

        ident = wp.tile([128, 128], F32, tag="ident")
        make_identity(nc, ident[:])
        identb = wp.tile([128, 128], BF16, tag="identb")
        nc.vector.tensor_copy(identb[:], ident[:])
        w2b, c1b, c2b, w1cb, n1mb = [], [], [], [], []
        for l in range(NL):
            w2b.append(wp.tile([128, 128], BF16, tag=f"w2b{l}", name=f"w2b{l}"))
            c1b.append(wp.tile([128, 128], BF16, tag=f"c1b{l}", name=f"c1b{l}"))
            c2b.append(wp.tile([128, 4], BF16, tag=f"c2b{l}", name=f"c2b{l}"))
            w1cb.append(wp.tile([128, 32], BF16, tag=f"w1cb{l}", name=f"w1cb{l}"))
            n1mb.append(wp.tile([32, 32], BF16, tag=f"n1mb{l}", name=f"n1mb{l}"))
        for l in range(NL):
            nc.vector.tensor_copy(w2b[l][:], W(f"W2{l}"))
            nc.vector.tensor_copy(c1b[l][:], W(f"C1{l}"))
            nc.vector.tensor_copy(c2b[l][:], W(f"C2{l}"))
            nc.vector.tensor_copy(w1cb[l][:], W(f"w1c{l}"))
            nc.vector.tensor_copy(n1mb[l][:], W(f"N1m{l}"))

        invc = wp.tile([128, BBLK], F32, tag="invc")
        nc.sync.dma_start(invc[:], ins["invc"][:])
        pselb = wp.tile([128, TOT], BF16, tag="pselb")
        maskb = wp.tile([128, TOT], BF16, tag="maskb")
        with tc.tile_pool(name="ld", bufs=1) as ld:
            pself = ld.tile([128, TOT], F32, tag="pself")
            nc.sync.dma_start(pself[:], ins["pself"][:])
            nc.vector.tensor_copy(pselb[:], pself[:])
            maskf = ld.tile([128, TOT], F32, tag="maskf")
            nc.sync.dma_start(maskf[:], ins["maskf"][:])
            nc.vector.tensor_copy(maskb[:], maskf[:])
        epst = wp.tile([128, 1], F32, tag="epst")
        nc.vector.memset(epst[:], 1e-12)

        hT = wp.tile([33, RPAD], F32, tag="hT")
        nc.vector.memset(hT[32:33, :], 1.0)
        coord = wp.tile([128, BBLK, 4], F32, tag="coord")
        nc.sync.dma_start(coord[:], ins["posnm"][:].rearrange("p (b c) -> p b c", c=4))
        aggT = wp.tile([32, RPAD], BF16, tag="aggT")
        tsnm = wp.tile([128, BBLK, 4], F32, tag="tsnm")
        A_all = wp.tile([128, BBLK, 32], BF16, tag="A_all")

        gcp = ctx.enter_context(tc.tile_pool(name="gcp", bufs=int(os.environ.get("EGNN_GCB", "5"))))
        ixp = ctx.enter_context(tc.tile_pool(name="ixp", bufs=4))
        ep = ctx.enter_context(tc.tile_pool(name="ep", bufs=2))
        tp = ctx.enter_context(tc.tile_pool(name="tp", bufs=2))
        pse = ctx.enter_context(tc.tile_pool(name="pse", bufs=1, space="PSUM"))
        psm = ctx.enter_context(tc.tile_pool(name="psm", bufs=1, space="PSUM"))
        psphi = ctx.enter_context(tc.tile_pool(name="psphi", bufs=1, space="PSUM"))
        psn = ctx.enter_context(tc.tile_pool(name="psn", bufs=1, space="PSUM"))

        # expand compact idx [16, TOT*8] -> idxfull [128, TOT*8] (DRAM->DRAM)
        for rep in range(8):
            nc.sync.dma_start(idxfull[16 * rep:16 * rep + 16, :], ins["idxc"][:])

        # zero pad rows of tabB once
        zt = wp.tile([128, 32], F32, tag="zt")
        nc.vector.memset(zt[:], 0.0)
        if NPAD > cfg.N:
            nc.sync.dma_start(tabB[cfg.N:NPAD, :], zt[0:NPAD - cfg.N, :])

        # h = emb_in(x)
        for o, w in CHUNKS:
            sl = slice(o, o + w)
            x2c = tp.tile([2, 512], F32, tag="x2c")
            nc.sync.dma_start(x2c[:, 0:w], ins["x2"][:, sl])
            ps = psn.tile([32, 512], F32, tag="psn")
            nc.tensor.matmul(ps[:, 0:w], lhsT=W("EMB"), rhs=x2c[:, 0:w])
            nc.scalar.copy(hT[0:32, sl], ps[:, 0:w])

        qctr = [0]

        def tab_block(lw, b):
            bsl = slice(b * 128, (b + 1) * 128)
            ps = psn.tile([128, 64], F32, tag="psn")
            nc.tensor.matmul(ps[:], lhsT=hT[:, bsl], rhs=W(f"AB{lw}"))
            nc.scalar.copy(A_all[:, b, :], ps[:, 0:32])
            tbb = tp.tile([128, 64], BF16, tag="tbb")
            nc.vector.tensor_copy(tbb[:, 0:32], ps[:, 32:64])
            tbf = tbb[:].bitcast(F32)  # [128, 32]
            nc.vector.tensor_copy(tbf[:, 16:19], coord[:, b, 0:3])
            nc.vector.memset(tbf[:, 19:32], 0.0)
            nc.sync.dma_start(
                tabBo[:].rearrange("(b p) c -> p b c", p=128)[:, b, :],
                tbf[:])

        def do_allgather():
            if not os.environ.get("EGNN_NOAG"):
                nc.gpsimd.collective_compute(
                    "AllGather", ALU.bypass,
                    replica_groups=[list(range(NC))],
                    ins=[tabBo[0:cfg.ROWN, :]],
                    outs=[tabB[0:cfg.N, :]],
                )

        def node_chunk(lw, o, w):
            sl = slice(o, o + w)
            ps1 = psn.tile([32, 512], F32, tag="psn")
            nc.tensor.matmul(ps1[:, 0:w], lhsT=W(f"N1h{lw}"), rhs=hT[:, sl],
                             start=True, stop=False)
            nc.tensor.matmul(ps1[:, 0:w], lhsT=n1mb[lw][:],
                             rhs=aggT[0:32, sl], start=False, stop=True)
            n1 = tp.tile([33, 512], F32, tag="n1")
            nc.vector.memset(n1[32:33, 0:w], 1.0)
            nc.scalar.activation(n1[0:32, 0:w], ps1[:, 0:w], ACTF.Silu)
            ps2 = psn.tile([32, 512], F32, tag="psn2")
            nc.tensor.matmul(ps2[:, 0:w], lhsT=W(f"N2{lw}"), rhs=n1[:, 0:w])
            nc.vector.tensor_tensor(hT[0:32, sl], hT[0:32, sl],
                                    ps2[:, 0:w], op=ALU.add)

        # ================= layers =================
        for l in range(NL):
            # ---- tab write + AllGather: only layer 0 does this up-front;
            # later layers fold node+tab into the previous edge loop ----
            if l == 0:
                for b in range(BBLK):
                    tab_block(l, b)
                do_allgather()

            # ---- edge phase: software-pipelined, gathers issued PRE ahead --
            PRE = int(os.environ.get("EGNN_PRE", "4"))
            gcs = {}

            def issue(b):
                S = S_list[b]
                gc = gcp.tile([128, S, 64], F32, tag="gc")
                if os.environ.get("EGNN_NOGATHER"):
                    nc.vector.memset(gc[:, 0:1, :], 1.0)
                else:
                    ixt = ixp.tile([128, S * 8], I16, tag="ixt")
                    nc.sync.dma_start(
                        ixt[:], idxfull[:, CUM[b] * 8:CUM[b] * 8 + S * 8])
                    # two half-gathers on different queues: halves the
                    # data-ready latency and doubles queue occupancy
                    nh = int(os.environ.get("EGNN_NH", "4"))
                    bnd = sorted({min(S, ((S * i // nh) + 3) // 4 * 4)
                                  for i in range(nh + 1)} | {0, S})
                    halves = [(bnd[i], bnd[i + 1]) for i in range(len(bnd) - 1)
                              if bnd[i] < bnd[i + 1]]
                    for (h0, h1) in halves:
                        nc.gpsimd.dma_gather(
                            out_ap=gc[:, h0:h1, :], in_ap=gsrc,
                            idxs_ap=ixt[:, h0 * 8:h1 * 8],
                            num_idxs=(h1 - h0) * 128,
                            num_idxs_reg=(h1 - h0) * 128, elem_size=64,
                            single_packet=False, queue_num=qctr[0] % NQ)
                        qctr[0] += 1
                gcs[b] = gc

            def compute(b):
                S = S_list[b]
                S4 = S // 4
                bsl = slice(b * 128, (b + 1) * 128)
                csl = slice(CUM[b], CUM[b] + S)
                gc = gcs.pop(b)
                gcb = gc[:].bitcast(BF16)  # [128, S, 128]
                if os.environ.get("EGNN_GATHERONLY"):
                    nc.vector.tensor_tensor(tsnm[:, b, 0:4], gc[:, 0, 0:4],
                                            gc[:, 1 % S, 0:4], op=ALU.add)
                    return

                psl3 = AP(pselb.tensor, pselb[:, csl].offset,
                          [pselb[:].ap[0], [1, S], [0, 3]])
                mskS = maskb[:, csl]

                # coord select (f32): cc = lo + parity*(hi-lo), folded into dif
                dcc = ep.tile([128, S, 3], F32, tag="dcc")
                nc.vector.tensor_tensor(dcc[:], gc[:, :, 48:51],
                                        gc[:, :, 16:19], op=ALU.subtract)
                nc.vector.tensor_tensor(dcc[:], dcc[:], psl3, op=ALU.mult)

                # B select (bf16): Bp = parity * (hi - lo)
                dB = ep.tile([128, S, 32], BF16, tag="dB")
                nc.vector.tensor_tensor(dB[:], gcb[:, :, 64:96],
                                        gcb[:, :, 0:32], op=ALU.subtract)
                Bp = ep.tile([128, S, 32], BF16, tag="Bp")
                nc.vector.tensor_tensor(
                    Bp[:], dB[:],
                    AP(pselb.tensor, pselb[:, csl].offset,
                       [pselb[:].ap[0], [1, S], [0, 32]]),
                    op=ALU.mult)

                # radial pipeline (edge-major [128, S, 3])
                crb = AP(coord.tensor, coord[:, b, 0:3].offset,
                         [coord[:].ap[0], [0, S], [1, 3]])
                dif = ep.tile([128, S, 3], F32, tag="dif")
                nc.vector.tensor_tensor(dif[:], crb, gc[:, :, 16:19],
                                        op=ALU.subtract)
                nc.vector.tensor_tensor(dif[:], dif[:], dcc[:], op=ALU.subtract)
                gq = ep.tile([128, S, 3], F32, tag="gq")
                nc.vector.tensor_scalar(gq[:], dif[:], 0.5 * L, None,
                                        op0=ALU.is_gt)
                tl = ep.tile([128, S, 3], F32, tag="tl")
                nc.vector.tensor_scalar(tl[:], dif[:], -0.5 * L, None,
                                        op0=ALU.is_lt)
                nc.vector.scalar_tensor_tensor(
                    dif[:], in0=gq[:], scalar=-L, in1=dif[:],
                    op0=ALU.mult, op1=ALU.add)
                nc.vector.scalar_tensor_tensor(
                    dif[:], in0=tl[:], scalar=L, in1=dif[:],
                    op0=ALU.mult, op1=ALU.add)
                sq = ep.tile([128, S, 3], F32, tag="sq")
                nc.vector.tensor_tensor(sq[:], dif[:], dif[:], op=ALU.mult)
                rad = ep.tile([128, S], F32, tag="rad")
                nc.vector.tensor_reduce(
                    rad[:], sq[:], axis=mybir.AxisListType.X, op=ALU.add)
                srt = ep.tile([128, S], F32, tag="srt")
                nc.scalar.activation(srt[:], rad[:],
                                     ACTF.Silu if os.environ.get("EGNN_FAKESQRT")
                                     else ACTF.Sqrt, bias=epst[:, :])
                rs = ep.tile([128, S], F32, tag="rs")
                nc.vector.reciprocal(rs[:], srt[:])
                radb = ep.tile([128, S], BF16, tag="radb")
                nc.scalar.copy(radb[:], rad[:])
                cdif = ep.tile([128, S, 3], F32, tag="cdif")
                nc.vector.tensor_tensor(
                    cdif[:], dif[:],
                    AP(rs.tensor, rs[:].offset, [rs[:].ap[0], [1, S], [0, 3]]),
                    op=ALU.mult)

                # m1 = w1c*rad + A[row] + B_lo + Bp   (bf16)
                m1 = ep.tile([128, S, 32], BF16, tag="m1")
                nc.vector.tensor_tensor(
                    m1[:],
                    AP(w1cb[l].tensor, w1cb[l][:].offset,
                       [w1cb[l][:].ap[0], [0, S], [1, 32]]),
                    AP(radb.tensor, radb[:].offset,
                       [radb[:].ap[0], [1, S], [0, 32]]),
                    op=ALU.mult)
                nc.vector.tensor_tensor(
                    m1[:], m1[:],
                    AP(A_all.tensor, A_all[:, b, :].offset,
                       [A_all[:].ap[0], [0, S], [1, 32]]),
                    op=ALU.add)
                nc.vector.tensor_tensor(m1[:], m1[:], gcb[:, :, 0:32],
                                        op=ALU.add)
                nc.vector.tensor_tensor(m1[:], m1[:], Bp[:], op=ALU.add)
                m1s = ep.tile([128, S, 32], BF16, tag="m1s")
                nc.scalar.activation(m1s[:], m1[:], ACTF.Silu)

                # MLP chain in batches of 4 groups (16 slots, 512 cols)
                vals = tp.tile([128, S, 32], BF16, tag="vals")
                phps = psphi.tile([128, SMAX], F32, tag="phps")
                NB = (S4 + 3) // 4
                for nb in range(NB):
                    g0 = nb * 4
                    ng = min(4, S4 - g0)
                    cols = ng * 128
                    t1 = pse.tile([128, 512], BF16, tag="t1")
                    for gi in range(ng):
                        nc.tensor.transpose(
                            t1[:, gi * 128:(gi + 1) * 128],
                            m1s[:, (g0 + gi) * 4:(g0 + gi) * 4 + 4, :],
                            identb[:])
                    m1sT = tp.tile([128, 512], BF16, tag="m1sT")
                    nc.scalar.copy(m1sT[:, 0:cols], t1[:, 0:cols])
                    mm = psm.tile([128, 512], F32, tag="mm")
                    nc.tensor.matmul(mm[:, 0:cols], lhsT=w2b[l][:],
                                     rhs=m1sT[:, 0:cols])
                    mT = tp.tile([128, 512], BF16, tag="mT")
                    nc.scalar.activation(mT[:, 0:cols], mm[:, 0:cols],
                                         ACTF.Silu, bias=W(f"b2{l}"))
                    pp = psm.tile([128, 512], F32, tag="pp")
                    nc.tensor.matmul(pp[:, 0:cols], lhsT=c1b[l][:],
                                     rhs=mT[:, 0:cols])
                    p1T = tp.tile([128, 512], BF16, tag="p1T")
                    nc.scalar.activation(p1T[:, 0:cols], pp[:, 0:cols],
                                         ACTF.Silu, bias=W(f"c1{l}"))
                    t2 = pse.tile([128, 512], BF16, tag="t2")
                    for gi in range(ng):
                        nc.tensor.matmul(
                            phps[:, (g0 + gi) * 4:(g0 + gi) * 4 + 4],
                            lhsT=p1T[:, gi * 128:(gi + 1) * 128],
                            rhs=c2b[l][:, 0:4])
                        nc.tensor.transpose(
                            t2[:, gi * 128:(gi + 1) * 128],
                            mT[:, gi * 128:(gi + 1) * 128], identb[:])
                    nc.vector.tensor_copy(
                        vals[:, g0 * 4:g0 * 4 + ng * 4, :],
                        t2[:, 0:cols].rearrange("p (g c) -> p g c", c=32))

                phi = ep.tile([128, S], F32, tag="phi")
                nc.scalar.activation(phi[:], phps[:, 0:S], ACTF.Tanh,
                                     bias=W(f"c2{l}"))
                phim = ep.tile([128, S], F32, tag="phim")
                nc.vector.tensor_tensor(phim[:], phi[:], mskS, op=ALU.mult)
                trans = ep.tile([128, S, 3], F32, tag="trans")
                nc.vector.tensor_tensor(
                    trans[:], cdif[:],
                    AP(phim.tensor, phim[:].offset,
                       [phim[:].ap[0], [1, S], [0, 3]]),
                    op=ALU.mult)
                # tsum: reduce over slots (view [128, 3, S])
                nc.vector.tensor_reduce(
                    tsnm[:, b, 0:3],
                    AP(trans.tensor, trans[:].offset,
                       [trans[:].ap[0], [1, 3], [3, S]]),
                    axis=mybir.AxisListType.X, op=ALU.add)
                # magg: mask then reduce over slots (view [128, 32, S])
                nc.vector.tensor_tensor(
                    vals[:], vals[:],
                    AP(maskb.tensor, maskb[:, csl].offset,
                       [maskb[:].ap[0], [1, S], [0, 32]]),
                    op=ALU.mult)
                magg = ep.tile([128, 32], F32, tag="magg")
                nc.vector.tensor_reduce(
                    magg[:],
                    AP(vals.tensor, vals[:].offset,
                       [vals[:].ap[0], [1, 32], [32, S]]),
                    axis=mybir.AxisListType.X, op=ALU.add)
                tmg = psn.tile([32, 128], F32, tag="tmg")
                nc.tensor.transpose(tmg[:], magg[:], ident[:])
                nc.scalar.copy(aggT[0:32, bsl], tmg[:])

            for i in range(BBLK + PRE):
                if i < BBLK:
                    issue(i)
                if i >= PRE:
                    b = i - PRE
                    compute(b)
                    # per-block coord update (tsum/cnt + residual)
                    nc.vector.tensor_tensor(
                        tsnm[:, b, 0:3], tsnm[:, b, 0:3],
                        AP(invc.tensor, invc[:, b].offset,
                           [invc[:].ap[0], [1, 1], [0, 3]]),
                        op=ALU.mult)
                    nc.vector.tensor_tensor(coord[:, b, 0:3], coord[:, b, 0:3],
                                            tsnm[:, b, 0:3], op=ALU.add)
                    # node MLP + next layer's tab rows as soon as a 512-col
                    # chunk of aggT completes (hidden under gather pipeline)
                    for (o, w) in CHUNKS:
                        if b == (o + w) // 128 - 1:
                            node_chunk(l, o, w)
                            if l + 1 < NL:
                                for b2 in range(o // 128, (o + w) // 128):
                                    tab_block(l + 1, b2)
            if l + 1 < NL:
                do_allgather()


        # ================= final =================
        for o, w in CHUNKS:
            sl = slice(o, o + w)
            ps = psn.tile([32, 512], F32, tag="psn")
            nc.tensor.matmul(ps[:, 0:w], lhsT=W("EOUT"), rhs=hT[:, sl])
            nc.scalar.copy(aggT[0:32, sl], ps[:, 0:w])
        psg = psphi.tile([32, G], F32, tag="phps")
        for b in range(BBLK):
            t = psn.tile([128, 32], BF16, tag="tmg")
            nc.tensor.transpose(t[:], aggT[0:32, b * 128:(b + 1) * 128],
                                identb[0:32, 0:32])
            onm = tp.tile([128, 32], BF16, tag="onm")
            nc.scalar.copy(onm[:], t[:])
            gohb = tp.tile([128, G], F32, tag="gohb")
            nc.sync.dma_start(gohb[:], ins["goh"][:, b * G:(b + 1) * G])
            gohbb = tp.tile([128, G], BF16, tag="gohbb")
            nc.vector.tensor_copy(gohbb[:], gohb[:])
            nc.tensor.matmul(psg[:], lhsT=onm[:], rhs=gohbb[:],
                             start=(b == 0), stop=(b == BBLK - 1))
        gsb = wp.tile([32, G], F32, tag="gsb")
        nc.scalar.copy(gsb[:], psg[:])
        nc.sync.dma_start(gsin[:], gsb[:])
        nc.gpsimd.collective_compute(
            "AllReduce", ALU.add, replica_groups=[list(range(NC))],
            ins=[gsin[:]], outs=[gsout[:]],
        )
        hg = wp.tile([33, G], F32, tag="hg")
        nc.vector.memset(hg[32:33, :], 1.0)
        nc.sync.dma_start(hg[0:32, :], gsout[:])
        pm = psn.tile([32, G], F32, tag="psn")
        nc.tensor.matmul(pm[:], lhsT=W("ZMU"), rhs=hg[:])
        msb = wp.tile([32, G], F32, tag="msb")
        nc.scalar.copy(msb[:], pm[:])
        nc.sync.dma_start(mu_o[:], msb[:])
        ps2 = psn.tile([32, G], F32, tag="psn")
        nc.tensor.matmul(ps2[:], lhsT=W("ZSIG"), rhs=hg[:])
        # softplus(x) = max(x,0) + ln(1 + exp(-|x|))
        zsb = wp.tile([32, G], F32, tag="zsb")
        nc.scalar.copy(zsb[:], ps2[:])
        axp = wp.tile([32, G], F32, tag="axp")
        nc.scalar.activation(axp[:], zsb[:], ACTF.Abs)
        nc.scalar.activation(axp[:], axp[:], ACTF.Exp, scale=-1.0)
        nc.vector.tensor_scalar_add(axp[:], axp[:], 1.0)
        nc.scalar.activation(axp[:], axp[:], ACTF.Ln)
        ssb = wp.tile([32, G], F32, tag="ssb")
        nc.vector.tensor_scalar(ssb[:], zsb[:], 0.0, None, op0=ALU.max)
        nc.vector.tensor_tensor(ssb[:], ssb[:], axp[:], op=ALU.add)
        nc.sync.dma_start(sig_o[:], ssb[:])


# ----------------------------------------------------------------------
import os

_CACHE = {}


def kernel(**inputs):
    import numpy as np
    from concourse import bacc
    from concourse.tile import TileContext
    from concourse.bass_utils import run_bass_kernel_spmd

    N = int(inputs["x"].shape[0])
    E = int(inputs["edge_index"].shape[1])
    G = 64
    lval = float(np.asarray(inputs["l"]).reshape(-1)[0])

    cfg = Cfg(N=N, E=E, G=G, L=lval)
    per_core = preprocess(inputs, cfg)
    specs = input_specs(cfg)

    key = (N, E, G, cfg.TOT, tuple(cfg.S_list), cfg.WC)
    if key in _CACHE:
        nc = _CACHE[key]
    else:
        nc = bacc.Bacc("TRN2", target_bir_lowering=False, debug=False,
                       num_devices=NC, num_swdge_queues=NQ)
        ins = {}
        for k, (shape, dt) in specs.items():
            mdt = {np.int16: mybir.dt.int16, np.float32: mybir.dt.float32}[dt]
            ins[k] = nc.dram_tensor(k, list(shape), mdt, kind="ExternalInput").ap()
        outs = {
            "mu": nc.dram_tensor("mu", [32, G], F32, kind="ExternalOutput").ap(),
            "sig": nc.dram_tensor("sig", [32, G], F32, kind="ExternalOutput").ap(),
        }
        with TileContext(nc) as tc:
            build(tc, outs, ins, cfg)
        nc.compile()
        _CACHE[key] = nc

    in_maps = []
    for c in range(NC):
        m = {}
        for k, (shape, dt) in specs.items():
            a = np.ascontiguousarray(per_core[c][k]).astype(dt)
            assert a.shape == tuple(shape), (k, a.shape, shape)
            m[k] = a
        in_maps.append(m)

    res = run_bass_kernel_spmd(nc, in_maps, core_ids=list(range(NC)))
    r0 = res.results[0]
    mu = np.ascontiguousarray(r0["mu"].T)
    sigma = np.ascontiguousarray(r0["sig"].T)

    iters = int(os.environ.get("EGNN_BENCH", "0"))
    if iters > 0:
        _bench_pjrt(nc, in_maps, NC, iters)
    return mu, sigma


def _bench_pjrt(nc, in_maps, n_cores, iters):
    """Wall-clock the sharded NEFF execution with device-resident inputs."""
    import time
    import numpy as np
    import jax
    from jax.sharding import Mesh, PartitionSpec, NamedSharding
    from jax.experimental.shard_map import shard_map
    import concourse.mybir as mybir
    from concourse import bass2jax
    from concourse.bass2jax import _bass_exec_p, partition_id_tensor

    bass2jax.install_neuronx_cc_hook()
    partition_name = (nc.partition_id_tensor.name
                      if nc.partition_id_tensor else None)
    in_names, out_names, out_avals, zero_outs = [], [], [], []
    for alloc in nc.m.functions[0].allocations:
        if not isinstance(alloc, mybir.MemoryLocationSet):
            continue
        name = alloc.memorylocations[0].name
        if alloc.kind == "ExternalInput":
            if name != partition_name:
                in_names.append(name)
        elif alloc.kind == "ExternalOutput":
            shape = tuple(alloc.tensor_shape)
            dtype = mybir.dt.np(alloc.dtype)
            out_names.append(name)
            out_avals.append(jax.core.ShapedArray(shape, dtype))
            zero_outs.append(np.zeros(shape, dtype))
    n_params = len(in_names)
    all_in_names = list(in_names) + list(out_names)
    if partition_name is not None:
        all_in_names.append(partition_name)

    def _body(*args):
        operands = list(args)
        if partition_name is not None:
            operands.append(partition_id_tensor())
        outs = _bass_exec_p.bind(
            *operands, out_avals=tuple(out_avals),
            in_names=tuple(all_in_names), out_names=tuple(out_names),
            lowering_input_output_aliases=(),
            sim_require_finite=True, sim_require_nnan=True, nc=nc)
        return tuple(outs)

    devices = jax.devices()[:n_cores]
    mesh = Mesh(np.asarray(devices), ("core",))
    spec = PartitionSpec("core")
    fn = jax.jit(shard_map(_body, mesh=mesh,
                           in_specs=(spec,) * (n_params + len(out_names)),
                           out_specs=(spec,) * len(out_names),
                           check_rep=False), keep_unused=True)
    sh = NamedSharding(mesh, spec)
    concat_in = [
        jax.device_put(
            np.concatenate([np.asarray(in_maps[c][nm]) for c in range(n_cores)],
                           axis=0), sh)
        for nm in in_names
    ]
    concat_zero = [
        jax.device_put(np.zeros((n_cores * z.shape[0], *z.shape[1:]), z.dtype), sh)
        for z in zero_outs
    ]
    outs = fn(*concat_in, *concat_zero)
    jax.block_until_ready(outs)  # warm compile + first exec
    times = []
    for _ in range(iters):
        t0 = time.perf_counter()
        outs = fn(*concat_in, *concat_zero)
        jax.block_until_ready(outs)
        times.append(time.perf_counter() - t0)
    best = min(times)
    print(f"bench iters(s): {[f'{t:.4f}' for t in times]}")
    for k in (20, 100):
        t0 = time.perf_counter()
        for _ in range(k):
            outs = fn(*concat_in, *concat_zero)
        jax.block_until_ready(outs)
        dt = time.perf_counter() - t0
        print(f"pipelined k={k}: total={dt*1e3:.2f} ms marginal={dt/k*1e6:.0f} us")
    print(f"HW exec time: {int(best * 1e9)} ns")



# revision 29
# speedup vs baseline: 1.3150x; 1.3150x over previous
"""EGNN encoder kernel for Trainium2 (Bass/Tile), 8-core SPMD — v3.

v3 design (on top of v2's degree-major edge layout):
  - m1 assembled FEAT-MAJOR in PSUM by PE accumulation: A-term matmul
    (replicated-identity lhsT), raw gather-slice transpose-accumulates
    (B_lo and psel*(B_hi-B_lo) quads), and w1c x rad rank-1 matmuls.
    First Silu fused into the PSUM evacuation on ACT.
  - magg on PE: slot-summing matmuls (rep32sel lhsT) accumulated per
    block in PSUM; dup-pad edges (pads replicate slot 0 of each dest)
    corrected by aggT = psmagg - kcnt * m0T. No mask mult, no vals
    copy, no big DVE reduce, no back-transposes.
  - Periodic wrap via ADD_RANGE_WRAP custom DVE op (1 op/component,
    per-partition dest-coord shift).
  - Sqrt grouped (G_B=7 blocks) so ACT silu<->sqrt table reloads drop
    from 2/block to 2/group.
  - AllGather of the node table chunked (5 chunks/layer, chunk-major
    global node ids) and overlapped with the edge loop.
"""

import math
import os
from contextlib import ExitStack
from dataclasses import dataclass, field

import numpy as np

import concourse.bass as bass
import concourse.tile as tile
from concourse import mybir
from concourse.bass import AP
from concourse.masks import make_identity

F32 = mybir.dt.float32
BF16 = mybir.dt.bfloat16
I32 = mybir.dt.int32
I16 = mybir.dt.int16
ALU = mybir.AluOpType
ACTF = mybir.ActivationFunctionType

NC = 8        # cores
H = 32        # hidden
NL = 4        # layers
NQ = 4        # SWDGE gather queues
G_B = 6       # blocks per sqrt group
AGB = 10      # blocks per allgather chunk


@dataclass
class Cfg:
    N: int
    E: int
    G: int
    L: float = 10.0
    EPS: float = 1e-8
    ROWN: int = 0
    BBLK: int = 0
    RPAD: int = 0
    NPAD: int = 0
    S_list: list = field(default_factory=list)
    CUM: list = field(default_factory=list)
    TOT: int = 0
    CHUNKS: list = field(default_factory=list)
    AGC: list = field(default_factory=list)   # allgather chunks (lo_row, hi_row)
    wslots: dict = field(default_factory=dict)
    WC: int = 0

    def derive_static(self):
        self.ROWN = self.N // NC
        self.BBLK = (self.ROWN + 127) // 128
        self.RPAD = self.BBLK * 128
        self.NPAD = ((self.N + 127) // 128) * 128
        self.CHUNKS = []
        o = 0
        while o < self.RPAD:
            w = min(512, self.RPAD - o)
            self.CHUNKS.append((o, w))
            o += w
        # allgather chunks: AGB blocks each (in local rows, capped at ROWN)
        self.AGC = []
        b = 0
        while b < self.BBLK:
            b2 = min(b + AGB, self.BBLK)
            lo = b * 128
            hi = min(b2 * 128, self.ROWN)
            self.AGC.append((lo, hi))
            b = b2


# ---------------------------------------------------------------- host pre

def preprocess(inp, cfg: Cfg):
    cfg.derive_static()
    N, E, G = cfg.N, cfg.E, cfg.G
    ROWN, BBLK = cfg.ROWN, cfg.BBLK

    row = np.asarray(inp["edge_index"][0]).astype(np.int64)
    col = np.asarray(inp["edge_index"][1]).astype(np.int64)
    pos = np.asarray(inp["pos"]).astype(np.float32)
    x_in = np.asarray(inp["x"]).astype(np.float32).reshape(-1)
    batch = np.asarray(inp["batch"]).astype(np.int64)

    # ---- degree-sorted relabeling, interleaved across cores ----
    deg = np.bincount(row, minlength=N)
    order = np.argsort(-deg, kind="stable")      # rank k -> orig node id
    k = np.arange(N)
    locid_of_rank = (k % NC) * ROWN + (k // NC)
    perm = np.empty(N, np.int64)                 # orig -> local id (c*ROWN+r)
    perm[order] = locid_of_rank
    # global table row (chunk-major): for core c, local row r in AG chunk j:
    #   g = NC*lo_j + c*(hi_j-lo_j) + (r-lo_j)
    glo = np.zeros(ROWN, np.int64)
    for (lo, hi) in cfg.AGC:
        glo[lo:hi] = np.arange(lo, hi) - lo + NC * lo
    gsize = {lo: hi - lo for (lo, hi) in cfg.AGC}
    gof = np.zeros(ROWN, np.int64)               # per-core stride offset
    for (lo, hi) in cfg.AGC:
        gof[lo:hi] = hi - lo
    # perm_glob[orig] = global row
    c_of = perm // ROWN
    r_of = perm % ROWN
    perm_glob = glo[r_of] + c_of * gof[r_of]

    row_n = perm[row]
    col_g = perm_glob[col]
    pos_n = np.empty_like(pos)
    pos_n[perm] = pos
    x_n = np.empty_like(x_in)
    x_n[perm] = x_in
    batch_n = np.empty_like(batch)
    batch_n[perm] = batch
    deg_n = np.zeros(N, np.int64)
    deg_n[perm] = deg
    deg_sorted = deg[order]                      # descending

    S_list = []
    for b in range(BBLK):
        d = int(deg_sorted[min(b * 128 * NC, N - 1)])
        S_list.append(max(4, ((d + 3) // 4) * 4))
    cfg.S_list = S_list
    cfg.CUM = np.concatenate([[0], np.cumsum(S_list)]).astype(np.int64).tolist()
    cfg.TOT = int(cfg.CUM[-1])
    TOT = cfg.TOT

    # ---- per-edge slot assignment (global sort by dest, split per core) ----
    eorder = np.argsort(row_n, kind="stable")
    rs, cs = row_n[eorder], col_g[eorder]
    node_start = np.searchsorted(rs, np.arange(N))
    j_all = np.arange(E, dtype=np.int64) - node_start[rs]

    gcnt = np.bincount(batch, minlength=G).astype(np.float64)
    invg = (1.0 / np.maximum(gcnt, 1.0)).astype(np.float32)

    per_core = []
    for c in range(NC):
        lo = np.searchsorted(rs, c * ROWN)
        hi = np.searchsorted(rs, (c + 1) * ROWN)
        r_loc = rs[lo:hi] - c * ROWN
        c_gl = cs[lo:hi]
        jj = j_all[lo:hi]
        bb = r_loc // 128
        pp = r_loc % 128

        idxc = np.zeros((16, TOT * 8), np.int16)     # compact wrap16 idx
        pself = np.zeros((128, TOT), np.float32)     # parity (0/1)
        kcnt = np.zeros((128, BBLK), np.float32)     # pad count per dest
        for b in range(BBLK):
            S = S_list[b]
            m = bb == b
            jb, pb, cb = jj[m], pp[m], c_gl[m]
            assert jb.max(initial=-1) < S, (c, b, S, jb.max(initial=-1))
            idxv = np.zeros((S, 128), np.int64)
            pselv = np.zeros((S, 128), np.int64)
            idxv[jb, pb] = cb >> 1
            pselv[jb, pb] = cb & 1
            degcol = np.zeros(128, np.int64)
            np.add.at(degcol, pb, 1)
            # dup-pad: pad slots replicate slot 0 of the same dest
            padm = np.arange(S)[:, None] >= degcol[None, :]
            idxv = np.where(padm, idxv[0:1, :], idxv)
            pselv = np.where(padm, pselv[0:1, :], pselv)
            flat = idxv.reshape(-1).astype(np.int16)     # [S*128] j*128+p
            idxc[:, cfg.CUM[b] * 8:cfg.CUM[b + 1] * 8] = \
                flat.reshape(S * 8, 16).T
            pself[:, cfg.CUM[b]:cfg.CUM[b] + S] = pselv.T.astype(np.float32)
            kcnt[:, b] = (S - degcol).astype(np.float32)

        import ml_dtypes
        d = {}
        d["idxc"] = idxc
        d["pself"] = pself
        d["kcnt"] = kcnt
        # kcntT [32, BBLK*128]: col b*128+p = pad count of dest (b,p)
        d["kcntT"] = np.tile(kcnt.T.reshape(1, -1), (32, 1)).astype(
            ml_dtypes.bfloat16)
        cnt = deg_n[c * ROWN:(c + 1) * ROWN].astype(np.float64)
        inv = np.zeros(cfg.RPAD, np.float32)
        inv[:ROWN] = (1.0 / np.maximum(cnt, 1.0)).astype(np.float32)
        d["invc"] = inv.reshape(BBLK, 128).T.copy()
        p4 = np.zeros((cfg.RPAD, 4), np.float32)
        p4[:ROWN, :3] = pos_n[c * ROWN:(c + 1) * ROWN]
        d["posnm"] = p4.reshape(BBLK, 128, 4).transpose(1, 0, 2).reshape(
            128, BBLK * 4).copy()
        x2 = np.zeros((2, cfg.RPAD), np.float32)
        x2[0, :ROWN] = x_n[c * ROWN:(c + 1) * ROWN]
        x2[1, :] = 1.0
        d["x2"] = x2
        goh = np.zeros((cfg.RPAD, G), np.float32)
        bloc = batch_n[c * ROWN:(c + 1) * ROWN]
        goh[np.arange(ROWN), bloc] = invg[bloc]
        d["goh"] = goh.reshape(BBLK, 128, G).transpose(1, 0, 2).reshape(
            128, BBLK * G).astype(ml_dtypes.bfloat16)
        per_core.append(d)

    # ---- weight pack (shared across cores) --------------------------------
    slots = {}
    colp = [0]
    wrows = []

    def add(name, arr):
        arr = np.asarray(arr, np.float32)
        assert arr.ndim == 2 and arr.shape[0] <= 128
        slots[name] = (arr.shape[0], colp[0], arr.shape[1])
        colp[0] += arr.shape[1]
        wrows.append(arr)

    def blkdiag(w, n=4):
        kk, m = w.shape
        out = np.zeros((n * kk, n * m), np.float32)
        for i in range(n):
            out[i * kk:(i + 1) * kk, i * m:(i + 1) * m] = w
        return out

    g = lambda kname: np.asarray(inp[kname], np.float32)
    add("EMB", np.vstack([g("emb_in_w"), g("emb_in_b")[None, :]]))
    add("REPA", np.tile(np.eye(32, dtype=np.float32), (1, 4)))
    add("R32S", np.tile(np.eye(32, dtype=np.float32), (4, 1)))
    for l in range(NL):
        w1 = g("edge_w1")[l]; b1 = g("edge_b1")[l]
        ab = np.zeros((33, 64), np.float32)
        ab[:32, :32] = w1[0:32]; ab[32, :32] = b1
        ab[:32, 32:] = w1[32:64]
        add(f"AB{l}", ab)
        w1c = w1[64, :]                       # [32] radial row
        wcs = np.zeros((16, 512), np.float32)
        for gg in range(4):
            for s in range(4):
                wcs[4 * gg + s, gg * 128 + s * 32:gg * 128 + s * 32 + 32] = w1c
        add(f"W1CS{l}", wcs)
        add(f"W2{l}", blkdiag(g("edge_w2")[l]))
        add(f"b2{l}", np.tile(g("edge_b2")[l], 4)[:, None])
        add(f"C1{l}", blkdiag(g("coord_w1")[l]))
        add(f"c1{l}", np.tile(g("coord_b1")[l], 4)[:, None])
        add(f"C2{l}", blkdiag(g("coord_w2")[l]))
        add(f"c2{l}", np.full((128, 1), float(g("coord_b2")[l][0]), np.float32))
        add(f"N1h{l}", np.vstack([g("node_w1")[l][0:32], g("node_b1")[l][None, :]]))
        add(f"N1m{l}", g("node_w1")[l][32:64])
        add(f"N2{l}", np.vstack([g("node_w2")[l], g("node_b2")[l][None, :]]))
    add("EOUT", np.vstack([g("emb_out_w"), g("emb_out_b")[None, :]]))
    add("ZMU", np.vstack([g("zmu_w"), g("zmu_b")[None, :]]))
    add("ZSIG", np.vstack([g("zsig_w"), g("zsig_b")[None, :]]))
    cfg.WC = colp[0]
    cfg.wslots = slots
    wpack = np.zeros((128, cfg.WC), np.float32)
    c0 = 0
    for arr in wrows:
        wpack[:arr.shape[0], c0:c0 + arr.shape[1]] = arr
        c0 += arr.shape[1]

    for d in per_core:
        d["wpack"] = wpack
    return per_core


def input_specs(cfg: Cfg):
    import ml_dtypes
    BBLK, RPAD, G, TOT = cfg.BBLK, cfg.RPAD, cfg.G, cfg.TOT
    return {
        "idxc": ((16, TOT * 8), np.int16),
        "pself": ((128, TOT), np.float32),
        "kcnt": ((128, BBLK), np.float32),
        "kcntT": ((32, BBLK * 128), ml_dtypes.bfloat16),
        "invc": ((128, BBLK), np.float32),
        "posnm": ((128, BBLK * 4), np.float32),
        "x2": ((2, RPAD), np.float32),
        "goh": ((128, BBLK * G), ml_dtypes.bfloat16),
        "wpack": ((128, cfg.WC), np.float32),
    }


# ---------------------------------------------------------------- builder

def build(tc, outs, ins, cfg: Cfg):
    from concourse.dve_ops import ADD_RANGE_WRAP
    nc = tc.nc
    BBLK, RPAD, NPAD, G = cfg.BBLK, cfg.RPAD, cfg.NPAD, cfg.G
    CHUNKS, S_list, CUM, TOT = cfg.CHUNKS, cfg.S_list, cfg.CUM, cfg.TOT
    SMAX = max(S_list)
    L = cfg.L

    mu_o, sig_o = outs["mu"], outs["sig"]

    # Double-buffered node table: layer l gathers read parity l%2 while
    # layer l+1's tab writes + chunked AllGather fill parity (l+1)%2.
    # (Collective writes are not WAR-ordered against in-flight gathers.)
    tabBo = [nc.dram_tensor(f"tabBo{pr}", [RPAD, 32], F32, kind="Internal")
             for pr in range(2)]
    tabB = [nc.dram_tensor(f"tabB{pr}", [NPAD, 32], F32, kind="Internal",
                           addr_space="Shared") for pr in range(2)]
    idxfull = nc.dram_tensor("idxfull", [128, TOT * 8], I16, kind="Internal")
    gsin = nc.dram_tensor("gsin", [32, G], F32, kind="Internal")
    gsout = nc.dram_tensor("gsout", [32, G], F32, kind="Internal",
                           addr_space="Shared")

    # gather source view: two 128B rows per 256B element
    gsrc = [AP(tabB[pr], 0, [[64, NPAD // 2], [1, 64]]) for pr in range(2)]

    ctx = ExitStack()
    with ctx:
        wp = ctx.enter_context(tc.tile_pool(name="wp", bufs=1))

        # ---------------- persistent state ----------------
        wt = wp.tile([128, cfg.WC], F32, tag="wt")
        nc.sync.dma_start(wt[:], ins["wpack"][:])

        def W(name):
            p, c0, w = cfg.wslots[name]
            return wt[0:p, c0:c0 + w]

        ident = wp.tile([128, 128], F32, tag="ident")
        make_identity(nc, ident[:])
        identb = wp.tile([128, 128], BF16, tag="identb")
        nc.vector.tensor_copy(identb[:], ident[:])
        repab = wp.tile([32, 128], BF16, tag="repab")
        nc.vector.tensor_copy(repab[:], W("REPA"))
        r32sb = wp.tile([128, 32], BF16, tag="r32sb")
        nc.vector.tensor_copy(r32sb[:], W("R32S"))
        kcntb = wp.tile([32, BBLK * 128], BF16, tag="kcntb")
        nc.sync.dma_start(kcntb[:], ins["kcntT"][:])
        w2b, c1b, c2b, w1cs, n1mb = [], [], [], [], []
        for l in range(NL):
            w2b.append(wp.tile([128, 128], BF16, tag=f"w2b{l}", name=f"w2b{l}"))
            c1b.append(wp.tile([128, 128], BF16, tag=f"c1b{l}", name=f"c1b{l}"))
            c2b.append(wp.tile([128, 4], BF16, tag=f"c2b{l}", name=f"c2b{l}"))
            w1cs.append(wp.tile([16, 512], BF16, tag=f"w1cs{l}", name=f"w1cs{l}"))
            n1mb.append(wp.tile([32, 32], BF16, tag=f"n1mb{l}", name=f"n1mb{l}"))
        for l in range(NL):
            nc.vector.tensor_copy(w2b[l][:], W(f"W2{l}"))
            nc.vector.tensor_copy(c1b[l][:], W(f"C1{l}"))
            nc.vector.tensor_copy(c2b[l][:], W(f"C2{l}"))
            nc.vector.tensor_copy(w1cs[l][:], W(f"W1CS{l}"))
            nc.vector.tensor_copy(n1mb[l][:], W(f"N1m{l}"))

        invc = wp.tile([128, BBLK], F32, tag="invc")
        nc.sync.dma_start(invc[:], ins["invc"][:])
        pselb = wp.tile([128, TOT], BF16, tag="pselb")
        with tc.tile_pool(name="ld", bufs=1) as ld:
            pself = ld.tile([128, TOT], F32, tag="pself")
            nc.sync.dma_start(pself[:], ins["pself"][:])
            nc.vector.tensor_copy(pselb[:], pself[:])
        kcb = wp.tile([128, BBLK], F32, tag="kcb")
        nc.sync.dma_start(kcb[:], ins["kcnt"][:])
        gohall = wp.tile([128, BBLK * G], BF16, tag="gohall")
        nc.sync.dma_start(gohall[:], ins["goh"][:])
        epst = wp.tile([128, 1], F32, tag="epst")
        nc.vector.memset(epst[:], 1e-12)

        hT = wp.tile([33, RPAD], F32, tag="hT")
        nc.vector.memset(hT[32:33, :], 1.0)
        coord = wp.tile([128, BBLK, 4], F32, tag="coord")
        nc.sync.dma_start(coord[:], ins["posnm"][:].rearrange("p (b c) -> p b c", c=4))
        aggT = wp.tile([32, RPAD], BF16, tag="aggT")
        AT_all = wp.tile([32, RPAD], BF16, tag="AT_all")
        tsnm = wp.tile([128, BBLK, 4], F32, tag="tsnm")
        GSP = G_B * SMAX  # per-group slot span (upper bound)

        gcp = ctx.enter_context(tc.tile_pool(name="gcp", bufs=int(os.environ.get("EGNN_GCB", "4"))))
        ixp = ctx.enter_context(tc.tile_pool(name="ixp", bufs=4))
        ep = ctx.enter_context(tc.tile_pool(name="ep", bufs=3))
        grp = ctx.enter_context(tc.tile_pool(name="grp", bufs=2))
        tp = ctx.enter_context(tc.tile_pool(name="tp", bufs=2))
        pse = ctx.enter_context(tc.tile_pool(name="pse", bufs=3, space="PSUM"))
        psg = ctx.enter_context(tc.tile_pool(name="psg", bufs=1, space="PSUM"))
        psn = ctx.enter_context(tc.tile_pool(name="psn", bufs=1, space="PSUM"))

        # expand compact idx [16, TOT*8] -> idxfull [128, TOT*8] (DRAM->DRAM)
        for rep in range(8):
            nc.scalar.dma_start(idxfull[16 * rep:16 * rep + 16, :], ins["idxc"][:])

        # h = emb_in(x)
        for o, w in CHUNKS:
            sl = slice(o, o + w)
            x2c = tp.tile([2, 512], F32, tag="x2c", bufs=1)
            nc.sync.dma_start(x2c[:, 0:w], ins["x2"][:, sl])
            ps = psn.tile([32, 512], F32, tag="psn")
            nc.tensor.matmul(ps[:, 0:w], lhsT=W("EMB"), rhs=x2c[:, 0:w])
            nc.scalar.copy(hT[0:32, sl], ps[:, 0:w])

        qctr = [0]

        def tab_block(lw, b):
            """Write node-table rows for block b (B-proj + coords) and the
            feat-major A projection for layer lw."""
            pr = lw % 2
            bsl = slice(b * 128, (b + 1) * 128)
            psB = psn.tile([128, 32], F32, tag="psTAB")
            nc.tensor.matmul(psB[:], lhsT=hT[:, bsl], rhs=W(f"AB{lw}")[:, 32:64])
            psA = psn.tile([32, 128], F32, tag="psTAB", name="psA")
            nc.tensor.matmul(psA[:], lhsT=W(f"AB{lw}")[:, 0:32], rhs=hT[:, bsl])
            nc.vector.tensor_copy(AT_all[:, bsl], psA[:])
            tbb = tp.tile([128, 64], BF16, tag="tbb")
            nc.vector.tensor_copy(tbb[:, 0:32], psB[:])
            tbf = tbb[:].bitcast(F32)  # [128, 32]
            nc.vector.tensor_copy(tbf[:, 16:19], coord[:, b, 0:3])
            nc.vector.memset(tbf[:, 19:32], 0.0)
            nc.sync.dma_start(
                tabBo[pr][:].rearrange("(b p) c -> p b c", p=128)[:, b, :],
                tbf[:])

        def ag_chunk(lw, j):
            pr = lw % 2
            lo, hi = cfg.AGC[j]
            nc.gpsimd.collective_compute(
                "AllGather", ALU.bypass,
                replica_groups=[list(range(NC))],
                ins=[tabBo[pr][lo:hi, :]],
                outs=[tabB[pr][NC * lo:NC * hi, :]],
            )

        def node_chunk(lw, o, w):
            sl = slice(o, o + w)
            ps1 = psn.tile([32, 512], F32, tag="psn")
            nc.tensor.matmul(ps1[:, 0:w], lhsT=W(f"N1h{lw}"), rhs=hT[:, sl],
                             start=True, stop=False)
            nc.tensor.matmul(ps1[:, 0:w], lhsT=n1mb[lw][:],
                             rhs=aggT[0:32, sl], start=False, stop=True)
            n1 = tp.tile([33, 512], F32, tag="n1", bufs=1)
            nc.vector.memset(n1[32:33, 0:w], 1.0)
            nc.scalar.activation(n1[0:32, 0:w], ps1[:, 0:w], ACTF.Silu)
            ps2 = psn.tile([32, 512], F32, tag="psn", name="ps2")
            nc.tensor.matmul(ps2[:, 0:w], lhsT=W(f"N2{lw}"), rhs=n1[:, 0:w])
            nc.vector.tensor_tensor(hT[0:32, sl], hT[0:32, sl],
                                    ps2[:, 0:w], op=ALU.add)

        # ================= layers =================
        for l in range(NL):
            if l == 0:
                agj = 0
                for b in range(BBLK):
                    tab_block(l, b)
                    while agj < len(cfg.AGC) and (b + 1) * 128 >= cfg.AGC[agj][1]:
                        ag_chunk(l, agj)
                        agj += 1
                while agj < len(cfg.AGC):
                    ag_chunk(l, agj)
                    agj += 1

            PRE = int(os.environ.get("EGNN_PRE", "4"))
            gcs = {}

            def issue(b):
                S = S_list[b]
                gc = gcp.tile([128, S, 64], F32, tag="gc")
                ixt = ixp.tile([128, S * 8], I16, tag="ixt")
                nc.sync.dma_start(
                    ixt[:], idxfull[:, CUM[b] * 8:CUM[b] * 8 + S * 8])
                nh = int(os.environ.get("EGNN_NH", "1"))
                bnd = sorted({min(S, ((S * i // nh) + 3) // 4 * 4)
                              for i in range(nh + 1)} | {0, S})
                halves = [(bnd[i], bnd[i + 1]) for i in range(len(bnd) - 1)
                          if bnd[i] < bnd[i + 1]]
                for (h0, h1) in halves:
                    nc.gpsimd.dma_gather(
                        out_ap=gc[:, h0:h1, :], in_ap=gsrc[l % 2],
                        idxs_ap=ixt[:, h0 * 8:h1 * 8],
                        num_idxs=(h1 - h0) * 128,
                        num_idxs_reg=(h1 - h0) * 128, elem_size=64,
                        single_packet=False, queue_num=qctr[0] % NQ)
                    qctr[0] += 1
                gcs[b] = gc

            gtiles = {}

            def stage1(b):
                """Per-block: coord diff+wrap+radial, B select product,
                feat-major m1 on PE, MLP chain, magg, phi."""
                S = S_list[b]
                S4 = S // 4
                bsl = slice(b * 128, (b + 1) * 128)
                csl = slice(CUM[b], CUM[b] + S)
                g = b // G_B
                if b % G_B == 0:
                    rad_g = grp.tile([128, GSP], F32, tag="rad_g")
                    dif_g = grp.tile([128, GSP, 3], F32, tag="dif_g")
                    phi_g = grp.tile([128, GSP], F32, tag="phi_g")
                    gtiles[g] = (rad_g, dif_g, phi_g)
                rad_g, dif_g, phi_g = gtiles[g]
                lo = CUM[b] - CUM[g * G_B]   # group-local slot offset
                gc = gcs.pop(b)
                gcb = gc[:].bitcast(BF16)  # [128, S, 128]
                dbgblk = int(os.environ.get("EGNN_DBG_BLK", "-1"))
                if os.environ.get("EGNN_DBG") and l == 0 and b == dbgblk:
                    nc.sync.dma_start(
                        outs["dbg_gc"][:], gc[:].rearrange("p s c -> p (s c)"))

                psl3 = AP(pselb.tensor, pselb[:, csl].offset,
                          [pselb[:].ap[0], [1, S], [0, 3]])
                # coord select + wrap:
                #   dcc = c_hi - c_lo ; t = psel*dcc ; u = -t - c_lo
                #   dif = wrap(u + c_dest)  (ADD_RANGE_WRAP per component)
                dcc = ep.tile([128, S, 3], F32, tag="dcc")
                nc.vector.tensor_tensor(dcc[:], gc[:, :, 48:51],
                                        gc[:, :, 16:19], op=ALU.subtract)
                nc.vector.tensor_tensor(dcc[:], dcc[:], psl3, op=ALU.mult)
                uu = ep.tile([128, S, 3], F32, tag="uu")
                nc.vector.scalar_tensor_tensor(
                    uu[:], in0=dcc[:], scalar=-1.0, in1=gc[:, :, 16:19],
                    op0=ALU.mult, op1=ALU.subtract)
                difd = AP(dif_g.tensor, dif_g[:, lo, :].offset,
                          [dif_g[:].ap[0], [3, S], [1, 3]])
                for cc in range(3):
                    nc.vector._custom_dve(
                        ADD_RANGE_WRAP,
                        out=AP(dif_g.tensor, dif_g[:, lo, cc].offset,
                               [dif_g[:].ap[0], [3, S]]),
                        in0=AP(uu.tensor, uu[:, 0, cc].offset,
                               [uu[:].ap[0], [3, S]]),
                        s0=coord[:, b, cc:cc + 1],
                        s1=0.5 * L, imm2=L)
                sq = ep.tile([128, S, 3], F32, tag="sq")
                nc.vector.tensor_tensor(sq[:], difd, difd, op=ALU.mult)
                nc.vector.tensor_reduce(
                    rad_g[:, lo:lo + S], sq[:], axis=mybir.AxisListType.X,
                    op=ALU.add)

                # B parity select: Bp = B_lo + psel*(B_hi - B_lo), in place
                Bp = ep.tile([128, S, 32], BF16, tag="Bp", bufs=2)
                nc.vector.tensor_tensor(Bp[:], gcb[:, :, 64:96],
                                        gcb[:, :, 0:32], op=ALU.subtract)
                nc.vector.tensor_tensor(
                    Bp[:], Bp[:],
                    AP(pselb.tensor, pselb[:, csl].offset,
                       [pselb[:].ap[0], [1, S], [0, 32]]),
                    op=ALU.mult)
                nc.vector.tensor_tensor(Bp[:], Bp[:], gcb[:, :, 0:32],
                                        op=ALU.add)

                # MLP chain, feat-major batches of 4 groups (512 cols)
                phps = psg.tile([128, SMAX], F32, tag="blockps")
                psmagg = psg.tile([32, 128], F32, tag="psmagg")
                m0c = ep.tile([32, 128], BF16, tag="m0c")
                last = (l == NL - 1)
                NB = (S4 + 3) // 4
                for nb in range(NB):
                    g0 = nb * 4
                    ng = min(4, S4 - g0)
                    cols = ng * 128
                    rqp = psn.tile([16, 128], F32, tag="rqp")
                    nc.tensor.transpose(
                        rqp[0:ng * 4, :],
                        rad_g[:, lo + g0 * 4:lo + g0 * 4 + ng * 4], ident[:])
                    rqS = ep.tile([16, 128], BF16, tag="rqS")
                    nc.vector.tensor_copy(rqS[0:ng * 4, :], rqp[0:ng * 4, :])
                    m1T = pse.tile([128, 512], F32, tag="big", name="m1T")
                    nc.tensor.matmul(
                        m1T[:, 0:cols], lhsT=repab[:],
                        rhs=AP(AT_all.tensor, AT_all[:, bsl].offset,
                               [AT_all[:].ap[0], [0, ng], [1, 128]]),
                        start=True, stop=False)
                    for gi in range(ng):
                        osl = slice(gi * 128, gi * 128 + 128)
                        q0 = (g0 + gi) * 4
                        nc.tensor.matmul(m1T[:, osl],
                                         lhsT=Bp[:, q0:q0 + 4, :],
                                         rhs=identb[:],
                                         start=False, stop=False)
                        nc.tensor.matmul(m1T[:, osl],
                                         lhsT=w1cs[l][0:ng * 4,
                                                      gi * 128:gi * 128 + 128],
                                         rhs=rqS[0:ng * 4, :],
                                         start=False, stop=True)
                    m1sT = tp.tile([128, 512], BF16, tag="m1sT")
                    nc.scalar.activation(m1sT[:, 0:cols], m1T[:, 0:cols],
                                         ACTF.Silu)
                    mm = pse.tile([128, 512], F32, tag="big", name="mm")
                    nc.tensor.matmul(mm[:, 0:cols], lhsT=w2b[l][:],
                                     rhs=m1sT[:, 0:cols])
                    mT = tp.tile([128, 512], BF16, tag="mT")
                    nc.scalar.activation(mT[:, 0:cols], mm[:, 0:cols],
                                         ACTF.Silu, bias=W(f"b2{l}"))
                    if nb == 0:
                        nc.scalar.copy(m0c[:], mT[0:32, 0:128])
                    for gi in range(ng):
                        nc.tensor.matmul(
                            psmagg[:], lhsT=r32sb[:],
                            rhs=mT[:, gi * 128:gi * 128 + 128],
                            start=(nb == 0 and gi == 0),
                            stop=(nb == NB - 1 and gi == ng - 1))
                    if not last:
                        pp = pse.tile([128, 512], F32, tag="big", name="pp")
                        nc.tensor.matmul(pp[:, 0:cols], lhsT=c1b[l][:],
                                         rhs=mT[:, 0:cols])
                        p1T = tp.tile([128, 512], BF16, tag="p1T")
                        nc.scalar.activation(p1T[:, 0:cols], pp[:, 0:cols],
                                             ACTF.Silu, bias=W(f"c1{l}"))
                        for gi in range(ng):
                            nc.tensor.matmul(
                                phps[:, (g0 + gi) * 4:(g0 + gi) * 4 + 4],
                                lhsT=p1T[:, gi * 128:(gi + 1) * 128],
                                rhs=c2b[l][:, 0:4])

                if not last:
                    nc.scalar.activation(phi_g[:, lo:lo + S], phps[:, 0:S],
                                         ACTF.Tanh, bias=W(f"c2{l}"))
                # magg correction: aggT = psmagg - kcnt * m0
                tmk = ep.tile([32, 128], BF16, tag="tmk")
                nc.vector.tensor_tensor(tmk[:], m0c[:], kcntb[:, bsl],
                                        op=ALU.mult)
                nc.vector.tensor_tensor(aggT[0:32, bsl], psmagg[:], tmk[:],
                                        op=ALU.subtract)

            def stage2(g):
                """Per sqrt-group: rsqrt of radials, trans, tsum, coord.
                Pad slots replicate slot 0, so tsum subtracts kcnt*trans0."""
                b0 = g * G_B
                b1 = min(b0 + G_B, BBLK)
                span = CUM[b1] - CUM[b0]
                rad_g, dif_g, phi_g = gtiles.pop(g)
                srt = ep.tile([128, GSP], F32, tag="srt", bufs=2)
                nc.scalar.activation(srt[:, 0:span], rad_g[:, 0:span],
                                     ACTF.Sqrt, bias=epst[:, :])
                nc.vector.reciprocal(rad_g[:, 0:span], srt[:, 0:span])
                dsl = AP(dif_g.tensor, dif_g[:].offset,
                         [dif_g[:].ap[0], [3, span], [1, 3]])
                nc.vector.tensor_tensor(
                    dsl, dsl,
                    AP(rad_g.tensor, rad_g[:].offset,
                       [rad_g[:].ap[0], [1, span], [0, 3]]),
                    op=ALU.mult)
                nc.vector.tensor_tensor(
                    dsl, dsl,
                    AP(phi_g.tensor, phi_g[:].offset,
                       [phi_g[:].ap[0], [1, span], [0, 3]]),
                    op=ALU.mult)
                for b in range(b0, b1):
                    S = S_list[b]
                    lo = CUM[b] - CUM[b0]
                    nc.vector.tensor_reduce(
                        tsnm[:, b, 0:3],
                        AP(dif_g.tensor, dif_g[:, lo, :].offset,
                           [dif_g[:].ap[0], [1, 3], [3, S]]),
                        axis=mybir.AxisListType.X, op=ALU.add)
                    t0k = ep.tile([128, 3], F32, tag="t0k")
                    nc.vector.tensor_tensor(
                        t0k[:], dif_g[:, lo, 0:3],
                        AP(kcb.tensor, kcb[:, b].offset,
                           [kcb[:].ap[0], [1, 1], [0, 3]]),
                        op=ALU.mult)
                    nc.vector.scalar_tensor_tensor(
                        tsnm[:, b, 0:3], in0=t0k[:], scalar=-1.0,
                        in1=tsnm[:, b, 0:3], op0=ALU.mult, op1=ALU.add)
                    nc.vector.tensor_tensor(
                        tsnm[:, b, 0:3], tsnm[:, b, 0:3],
                        AP(invc.tensor, invc[:, b].offset,
                           [invc[:].ap[0], [1, 1], [0, 3]]),
                        op=ALU.mult)
                    nc.vector.tensor_tensor(coord[:, b, 0:3], coord[:, b, 0:3],
                                            tsnm[:, b, 0:3], op=ALU.add)

            NGRP = (BBLK + G_B - 1) // G_B
            nchunk = [0]   # next node chunk index
            agj = [0]      # next allgather chunk

            def after_stage2(g):
                b1 = min(g * G_B + G_B, BBLK)
                while nchunk[0] < len(CHUNKS):
                    o, w = CHUNKS[nchunk[0]]
                    if (o + w) // 128 > b1:
                        break
                    node_chunk(l, o, w)
                    if l + 1 < NL:
                        for b2 in range(o // 128, (o + w) // 128):
                            tab_block(l + 1, b2)
                        while (agj[0] < len(cfg.AGC)
                               and (o + w) >= cfg.AGC[agj[0]][1]):
                            ag_chunk(l + 1, agj[0])
                            agj[0] += 1
                    nchunk[0] += 1

            for i in range(BBLK + PRE):
                if i < BBLK:
                    issue(i)
                if i >= PRE:
                    b = i - PRE
                    stage1(b)
                    if b % G_B == G_B - 1 or b == BBLK - 1:
                        g = b // G_B
                        if l < NL - 1:
                            stage2(g)
                        else:
                            gtiles.pop(g)
                        after_stage2(g)
            if l + 1 < NL:
                while agj[0] < len(cfg.AGC):
                    ag_chunk(l + 1, agj[0])
                    agj[0] += 1
            if os.environ.get("EGNN_DBG") and l == 0:
                nc.sync.dma_start(outs["dbg_tab"][:], tabB[1][0:cfg.N, :])
                nc.sync.dma_start(outs["dbg_h"][:], hT[:])
                nc.sync.dma_start(outs["dbg_coord"][:],
                                  coord[:].rearrange("p b c -> p (b c)"))
                nc.sync.dma_start(outs["dbg_agg"][:], aggT[:])

        # ================= final =================
        psgg = psg.tile([32, 128], F32, tag="psmagg", name="psgg")[:, 0:G]
        for b in range(BBLK):
            bsl = slice(b * 128, (b + 1) * 128)
            ps = psn.tile([128, 32], F32, tag="psTAB", name="pso")
            nc.tensor.matmul(ps[:], lhsT=hT[:, bsl], rhs=W("EOUT"))
            onm = tp.tile([128, 32], BF16, tag="onm")
            nc.scalar.copy(onm[:], ps[:])
            nc.tensor.matmul(psgg[:], lhsT=onm[:],
                             rhs=gohall[:, b * G:(b + 1) * G],
                             start=(b == 0), stop=(b == BBLK - 1))
        gsb = wp.tile([32, G], F32, tag="gsb")
        nc.scalar.copy(gsb[:], psgg[:])
        nc.sync.dma_start(gsin[:], gsb[:])
        nc.gpsimd.collective_compute(
            "AllReduce", ALU.add, replica_groups=[list(range(NC))],
            ins=[gsin[:]], outs=[gsout[:]],
        )
        hg = wp.tile([33, G], F32, tag="hg")
        nc.vector.memset(hg[32:33, :], 1.0)
        nc.sync.dma_start(hg[0:32, :], gsout[:])
        pm = psn.tile([32, G], F32, tag="psn")
        nc.tensor.matmul(pm[:], lhsT=W("ZMU"), rhs=hg[:])
        msb = wp.tile([32, G], F32, tag="msb")
        nc.scalar.copy(msb[:], pm[:])
        nc.sync.dma_start(mu_o[:], msb[:])
        ps2 = psn.tile([32, G], F32, tag="psn")
        nc.tensor.matmul(ps2[:], lhsT=W("ZSIG"), rhs=hg[:])
        # softplus(x) = max(x,0) + ln(1 + exp(-|x|))
        zsb = wp.tile([32, G], F32, tag="zsb")
        nc.scalar.copy(zsb[:], ps2[:])
        axp = wp.tile([32, G], F32, tag="axp")
        nc.scalar.activation(axp[:], zsb[:], ACTF.Abs)
        nc.scalar.activation(axp[:], axp[:], ACTF.Exp, scale=-1.0)
        nc.vector.tensor_scalar_add(axp[:], axp[:], 1.0)
        nc.scalar.activation(axp[:], axp[:], ACTF.Ln)
        ssb = wp.tile([32, G], F32, tag="ssb")
        nc.vector.tensor_scalar(ssb[:], zsb[:], 0.0, None, op0=ALU.max)
        nc.vector.tensor_tensor(ssb[:], ssb[:], axp[:], op=ALU.add)
        nc.sync.dma_start(sig_o[:], ssb[:])


# ----------------------------------------------------------------------

_CACHE = {}


def kernel(**inputs):
    import numpy as np
    from concourse import bacc
    from concourse.tile import TileContext
    from concourse.bass_utils import run_bass_kernel_spmd

    N = int(inputs["x"].shape[0])
    E = int(inputs["edge_index"].shape[1])
    G = 64
    lval = float(np.asarray(inputs["l"]).reshape(-1)[0])

    cfg = Cfg(N=N, E=E, G=G, L=lval)
    per_core = preprocess(inputs, cfg)
    specs = input_specs(cfg)

    key = (N, E, G, cfg.TOT, tuple(cfg.S_list), cfg.WC,
           os.environ.get("EGNN_DBG", ""), os.environ.get("EGNN_DBG_BLK", ""))
    if key in _CACHE:
        nc = _CACHE[key]
    else:
        import ml_dtypes
        nc = bacc.Bacc("TRN2", target_bir_lowering=False, debug=False,
                       num_devices=NC, num_swdge_queues=NQ)
        ins = {}
        for k, (shape, dt) in specs.items():
            mdt = {np.int16: mybir.dt.int16, np.float32: mybir.dt.float32,
                   ml_dtypes.bfloat16: mybir.dt.bfloat16}[dt]
            ins[k] = nc.dram_tensor(k, list(shape), mdt, kind="ExternalInput").ap()
        outs = {
            "mu": nc.dram_tensor("mu", [32, G], F32, kind="ExternalOutput").ap(),
            "sig": nc.dram_tensor("sig", [32, G], F32, kind="ExternalOutput").ap(),
        }
        if os.environ.get("EGNN_DBG"):
            outs["dbg_h"] = nc.dram_tensor(
                "dbg_h", [33, cfg.RPAD], F32, kind="ExternalOutput").ap()
            outs["dbg_coord"] = nc.dram_tensor(
                "dbg_coord", [128, cfg.BBLK * 4], F32,
                kind="ExternalOutput").ap()
            outs["dbg_agg"] = nc.dram_tensor(
                "dbg_agg", [32, cfg.RPAD], mybir.dt.bfloat16,
                kind="ExternalOutput").ap()
            outs["dbg_tab"] = nc.dram_tensor(
                "dbg_tab", [cfg.N, 32], F32, kind="ExternalOutput").ap()
            dbgblk = int(os.environ.get("EGNN_DBG_BLK", "-1"))
            if dbgblk >= 0:
                outs["dbg_gc"] = nc.dram_tensor(
                    "dbg_gc", [128, cfg.S_list[dbgblk] * 64], F32,
                    kind="ExternalOutput").ap()
        with TileContext(nc) as tc:
            build(tc, outs, ins, cfg)
        nc.compile()
        _CACHE[key] = nc

    in_maps = []
    for c in range(NC):
        m = {}
        for k, (shape, dt) in specs.items():
            a = np.ascontiguousarray(per_core[c][k]).astype(dt)
            assert a.shape == tuple(shape), (k, a.shape, shape)
            m[k] = a
        in_maps.append(m)

    res = run_bass_kernel_spmd(nc, in_maps, core_ids=list(range(NC)))
    global _LAST
    _LAST = res.results
    r0 = res.results[0]
    mu = np.ascontiguousarray(r0["mu"].T)
    sigma = np.ascontiguousarray(r0["sig"].T)

    iters = int(os.environ.get("EGNN_BENCH", "0"))
    if iters > 0:
        _bench_pjrt(nc, in_maps, NC, iters)
    return mu, sigma


def _bench_pjrt(nc, in_maps, n_cores, iters):
    """Wall-clock the sharded NEFF execution with device-resident inputs."""
    import time
    import numpy as np
    import jax
    from jax.sharding import Mesh, PartitionSpec, NamedSharding
    from jax.experimental.shard_map import shard_map
    import concourse.mybir as mybir
    from concourse import bass2jax
    from concourse.bass2jax import _bass_exec_p, partition_id_tensor

    bass2jax.install_neuronx_cc_hook()
    partition_name = (nc.partition_id_tensor.name
                      if nc.partition_id_tensor else None)
    in_names, out_names, out_avals, zero_outs = [], [], [], []
    for alloc in nc.m.functions[0].allocations:
        if not isinstance(alloc, mybir.MemoryLocationSet):
            continue
        name = alloc.memorylocations[0].name
        if alloc.kind == "ExternalInput":
            if name != partition_name:
                in_names.append(name)
        elif alloc.kind == "ExternalOutput":
            shape = tuple(alloc.tensor_shape)
            dtype = mybir.dt.np(alloc.dtype)
            out_names.append(name)
            out_avals.append(jax.core.ShapedArray(shape, dtype))
            zero_outs.append(np.zeros(shape, dtype))
    n_params = len(in_names)
    all_in_names = list(in_names) + list(out_names)
    if partition_name is not None:
        all_in_names.append(partition_name)

    def _body(*args):
        operands = list(args)
        if partition_name is not None:
            operands.append(partition_id_tensor())
        outs = _bass_exec_p.bind(
            *operands, out_avals=tuple(out_avals),
            in_names=tuple(all_in_names), out_names=tuple(out_names),
            lowering_input_output_aliases=(),
            sim_require_finite=True, sim_require_nnan=True, nc=nc)
        return tuple(outs)

    devices = jax.devices()[:n_cores]
    mesh = Mesh(np.asarray(devices), ("core",))
    spec = PartitionSpec("core")
    fn = jax.jit(shard_map(_body, mesh=mesh,
                           in_specs=(spec,) * (n_params + len(out_names)),
                           out_specs=(spec,) * len(out_names),
                           check_rep=False), keep_unused=True)
    sh = NamedSharding(mesh, spec)
    concat_in = [
        jax.device_put(
            np.concatenate([np.asarray(in_maps[c][nm]) for c in range(n_cores)],
                           axis=0), sh)
        for nm in in_names
    ]
    concat_zero = [
        jax.device_put(np.zeros((n_cores * z.shape[0], *z.shape[1:]), z.dtype), sh)
        for z in zero_outs
    ]
    outs = fn(*concat_in, *concat_zero)
    jax.block_until_ready(outs)  # warm compile + first exec
    times = []
    for _ in range(iters):
        t0 = time.perf_counter()
        outs = fn(*concat_in, *concat_zero)
        jax.block_until_ready(outs)
        times.append(time.perf_counter() - t0)
    best = min(times)
    print(f"bench iters(s): {[f'{t:.4f}' for t in times]}")
    for k in (20, 100):
        t0 = time.perf_counter()
        for _ in range(k):
            outs = fn(*concat_in, *concat_zero)
        jax.block_until_ready(outs)
        dt = time.perf_counter() - t0
        print(f"pipelined k={k}: total={dt*1e3:.2f} ms marginal={dt/k*1e6:.0f} us")
    print(f"HW exec time: {int(best * 1e9)} ns")
